# revision 23
# baseline (speedup 1.0000x reference)
"""Trainium2 Bass kernel for nn_AConvCircular2D (4x128x48x48, 8 heads, dk=dv=128).

Sharding: core c handles batch b = c//2 and head-group g = c%2 (heads 4g..4g+3).
Per core:
  - 3x3 circular convs (init 64ch + q 64ch + k 64ch + v 64ch) as 9-tap matmul
    accumulation over a circularly padded x in SBUF (all bf16, fp32 accum).
  - attention, pair-interleaved, n-window (768) outer / m-tile inner:
      slab (pi, mt) = logitsT for heads {2pi, 2pi+1} on PE row-bands 32h;
      exp on ACT (pair 0) or DVE int16-bf16-bits fast-exp (pair 1), running
      concurrently; attn^T[d, n-window] += vt[mt]^T @ E with the small v^T
      chunk [128, 17] stationary (ones column -> softmax sums row).
      attnv is software-pipelined one mt behind logits so the PE never
      blocks on the exp engines. Conv pass 2 (init|v) and the v^T build are
      emitted AFTER the first logits slab so the exp stream starts early.
  - per-window finalize: DVE 32x32 block transposes PSUM->SBUF (scrambled
    layout), softmax divide (DVE recip + GpSimd mul), DMAs into
    channel-aligned DRAM chunks; one AllGather per chunk {w0: ch 0:4,
    w1: ch 4:8, w2: ch 8:16} across the core pair -> only the last is
    tail-exposed.
  - 1x1 out-conv (64 out channels per core).
Output per core: (128, 2304) f32 = [64 init-conv channels | 64 attn-conv channels].
"""

import sys

sys.path.insert(0, "/opt/trn_rl_repo")

import numpy as np

import concourse.bass as bass  # noqa: F401
import concourse.mybir as mybir
from concourse import bacc, tile
from concourse.bass_utils import run_bass_kernel_spmd

F32 = mybir.dt.float32
BF16 = mybir.dt.bfloat16
AF = mybir.ActivationFunctionType

H = 48
W = 48
N = H * W            # 2304
NT = N // 128        # 18 n-tiles / m-tiles of 128
DH = 16              # per-head dim
HL = 4               # heads per core
SCALE = DH ** -0.5   # 0.25
I16 = mybir.dt.int16

# DVE fast-exp: bf16 bit pattern of 2^(SCALE*log2e*x) built as int16
# bits = trunc(x * EXP_A + EXP_B); softmax cancels the constant-factor part
# of the approximation error, leaving only mantissa-curvature (~1.4% on a
# fully-converted softmax row; diluted by the 50/50 ACT/DVE pair split).
EXP_A = float(SCALE * 1.4426950408889634 * 128.0)
EXP_B = 16250.5


def build(DEBUG=False):
    nc = bacc.Bacc("TRN2", target_bir_lowering=False, debug=False)

    x_ext = nc.declare_dram_parameter("x", [128, H, W], F32, isOutput=False)
    wc_ext = nc.declare_dram_parameter("wc", [128, 9 * 256], F32, isOutput=False)
    wo_ext = nc.declare_dram_parameter("wo", [128, 64], F32, isOutput=False)
    out_ext = nc.declare_dram_parameter("out", [128, N], F32, isOutput=True)

    with tile.TileContext(nc) as tc:
        with (
            tc.tile_pool(name="const", bufs=1) as cp,
            tc.tile_pool(name="dram", bufs=1, space="DRAM") as dram,
        ):
            xp_bf = cp.tile([128, 50 * 50], BF16)    # circular-padded x
            wc_bf = cp.tile([128, 9 * 256], BF16)    # conv weights, tap-major
            wo_bf = cp.tile([128, 64], BF16)         # out-conv weights (in,out)
            qm_sb = cp.tile([128, N], BF16)          # [k_h | q_h] per 32-block
            kz_sb = cp.tile([128, N], BF16)          # [0 | k_h] per 32-block
            v_sb = cp.tile([128, N], BF16)           # rows 64..127 hold v (conv order)
            vv_sb = cp.tile([128, N], BF16)          # v_h at base 32h
            vt_sb = cp.tile([128, HL * NT * 17], BF16)  # v^T + ones col per m-tile
            g_sb = cp.tile([128, N], BF16)           # gathered attn channels
            id_bf = cp.tile([128, 16], BF16)  # identity replicated per 16-row block
            zrow = cp.tile([1, 1024], BF16)   # zeros for HAM heater bursts

            bounceA = dram.tile([HL, 9216], BF16)    # channels 0:4 per head (after w0)
            bounceB = dram.tile([HL, 9216], BF16)    # channels 4:8 (after w1)
            bounceC = dram.tile([HL, 18432], BF16)   # channels 8:16 (after w2)
            gathA = dram.tile([2, HL * 9216], BF16)
            gathB = dram.tile([2, HL * 9216], BF16)
            gathC = dram.tile([2, HL * 18432], BF16)

            # ---------------- load + prep + conv pass 1 (q|k) ----------------
            with (
                tc.tile_pool(name="stage", bufs=2) as stage,
                tc.tile_pool(name="ps_a", bufs=3, space="PSUM") as ps_a,
            ):
                # PE warm-up: dummy matmuls during input DMA to lift the HAM
                # clock gate to 2.4 GHz before the conv streams
                warm = stage.tile([128, 512], BF16, tag="warm")
                nc.vector.memset(warm[:], 0.0)
                wps = ps_a.tile([128, 512], F32, tag="wps", bufs=1)
                for _ in range(16):
                    nc.tensor.matmul(wps[:], warm[:, 0:128], warm[:], start=True, stop=True)
                # preload the exp ACT table set off the critical path
                wexp = stage.tile([128, 16], BF16, tag="wexp")
                nc.scalar.activation(wexp[:], warm[:, 0:16], AF.Exp)

                x_f32 = stage.tile([128, 2304], F32, tag="xstage")
                nc.sync.dma_start(x_f32[:], x_ext[:].rearrange("p y x -> p (y x)"))
                x2 = x_f32[:].rearrange("p (y x) -> p y x", y=48)
                xp2b = xp_bf[:].rearrange("p (y x) -> p y x", y=50)
                # halo build on DVE with fused f32->bf16 cast
                nc.vector.tensor_copy(xp2b[:, 1:49, 1:49], x2[:, :, :])
                nc.vector.tensor_copy(xp2b[:, 1:49, 0:1], x2[:, :, 47:48])
                nc.vector.tensor_copy(xp2b[:, 1:49, 49:50], x2[:, :, 0:1])
                nc.vector.tensor_copy(xp2b[:, 0:1, 1:49], x2[:, 47:48, :])
                nc.vector.tensor_copy(xp2b[:, 49:50, 1:49], x2[:, 0:1, :])
                nc.vector.tensor_copy(xp2b[:, 0:1, 0:1], x2[:, 47:48, 47:48])
                nc.vector.tensor_copy(xp2b[:, 0:1, 49:50], x2[:, 47:48, 0:1])
                nc.vector.tensor_copy(xp2b[:, 49:50, 0:1], x2[:, 0:1, 47:48])
                nc.vector.tensor_copy(xp2b[:, 49:50, 49:50], x2[:, 0:1, 0:1])

                wc_f32 = stage.tile([128, 9 * 256], F32, tag="wcstage")
                nc.sync.dma_start(wc_f32[:], wc_ext[:])
                nc.vector.tensor_copy(wc_bf[:], wc_f32[:])

                wo_f32 = stage.tile([128, 64], F32, tag="wo")
                nc.sync.dma_start(wo_f32[:], wo_ext[:])
                nc.vector.tensor_copy(wo_bf[:], wo_f32[:])

                nc.gpsimd.memset(id_bf[:], 0.0)
                nc.gpsimd.affine_select(
                    out=id_bf[0:16, :],
                    in_=id_bf[0:16, :],
                    compare_op=mybir.AluOpType.not_equal,
                    fill=1.0,
                    base=0,
                    pattern=[[-1, 16]],
                    channel_multiplier=1,
                )
                for blk in (2, 4, 6):
                    nc.sync.dma_start(id_bf[16 * blk : 16 * blk + 16, :], id_bf[0:16, :])
                nc.gpsimd.memset(vt_sb[:], 1.0)
                nc.gpsimd.memset(kz_sb[:], 0.0)
                nc.gpsimd.memset(zrow[:], 0.0)

                # conv pass 1: [k|q] channels
                xp2_bf = xp_bf[:].rearrange("p (y x) -> p y x", y=50)
                for j in range(6):  # n-chunks of 384 (8 rows)
                    p = ps_a.tile([128, 384], F32, tag="conv")
                    for t in range(9):
                        dy, dx = t // 3, t % 3
                        rhs = xp2_bf[:, 8 * j + dy : 8 * j + dy + 8, dx : dx + 48]
                        nc.tensor.matmul(
                            p[:],
                            wc_bf[:, t * 256 : t * 256 + 128],
                            rhs,
                            start=(t == 0),
                            stop=(t == 8),
                        )
                    nc.vector.tensor_copy(qm_sb[:, 384 * j : 384 * (j + 1)], p[:, :])

                # shift k into the [0|k] stationary layout
                for h in range(HL):
                    nc.sync.dma_start(
                        kz_sb[32 * h + 16 : 32 * h + 32, :], qm_sb[32 * h : 32 * h + 16, :]
                    )

            # ---------------- attention + conv pass 2, pipelined ----------------
            with (
                tc.tile_pool(name="epool", bufs=3) as epool,
                tc.tile_pool(name="apool", bufs=2) as apool,
                tc.tile_pool(name="ps_log", bufs=2, space="PSUM") as ps_log,
            ):
                def emit_slab(st, mt, heat=False):
                    """Emit both pairs' logits + exps for (st, mt); return E info."""
                    nb = 768 * st
                    es, fasts = [], []
                    for pi in range(2):
                        h0, h1 = 2 * pi, 2 * pi + 1
                        L = ps_log.tile([128, 1536], F32, tag="L", name="L")
                        if heat and pi == 0:
                            emit_heater_burst(L)
                        kz0 = kz_sb[32 * h0 : 32 * h0 + 32, 128 * mt : 128 * (mt + 1)]
                        kz1 = kz_sb[32 * h1 : 32 * h1 + 32, 128 * mt : 128 * (mt + 1)]
                        # issue order keeps the two bank-1 chunks (cols 512:768
                        # and 768:1024) temporally disjoint under strip concurrency
                        for (i, kzh, cols, ns, w_) in (
                            (0, kz0, 512, 512, 256),
                            (1, kz1, 1024, 256, 512),
                            (0, kz0, 0, 0, 512),
                            (1, kz1, 768, 0, 256),
                        ):
                            h = 2 * pi + i
                            nc.tensor.matmul(
                                L[:, cols : cols + w_],
                                kzh,
                                qm_sb[32 * h : 32 * h + 32, nb + ns : nb + ns + w_],
                                start=True,
                                stop=True,
                                tile_position=(32 * h, 0),
                            )
                        # split the slab's exp across ACT and DVE (concurrent):
                        # ACT gets 896 cols, DVE 640 via the int16 bits trick;
                        # sides alternate per mt so every softmax row mixes both.
                        e = epool.tile([128, 1536], BF16, tag="E", name="E")
                        if mt % 2 == 0:
                            a0, a1, d0, d1 = 0, 896, 896, 1536
                        else:
                            d0, d1, a0, a1 = 0, 640, 640, 1536
                        nc.scalar.activation(
                            e[:, a0:a1], L[:, a0:a1], AF.Exp, scale=SCALE
                        )
                        nc.vector.tensor_scalar(
                            e[:, d0:d1].bitcast(I16), L[:, d0:d1], EXP_A, EXP_B,
                            op0=mybir.AluOpType.mult,
                            op1=mybir.AluOpType.add,
                        )
                        es.append(e)
                    return es, fasts

                def emit_attnv(acc, st, mt, es, fasts):
                    first, last = (mt == 0), (mt == NT - 1)
                    for pi in range(2):
                        e = es[pi]
                        for i in range(2):
                            h = 2 * pi + i
                            vt = vt_sb[:, (h * NT + mt) * 17 : (h * NT + mt) * 17 + 17]
                            for (c0, w_) in ((0, 512), (512, 256)):
                                rhs = e[:, 768 * i + c0 : 768 * i + c0 + w_]
                                nc.tensor.matmul(
                                    acc[32 * h : 32 * h + 17, c0 : c0 + w_],
                                    vt,
                                    rhs,
                                    start=first,
                                    stop=last,
                                    tile_position=(0, 32 * h),
                                    skip_group_check=True,
                                )

                def emit_finalize(acc, st):
                    # One 32x32-block DVE transpose per head band, PSUM->SBUF:
                    # scr[32h+a, 128t+32c+b] = acc[32h+b, 128t+32c+a], i.e.
                    # partition a = n%32, col b = d-index (0:16 data, 16 sums).
                    # The bounce DMAs' APs unscramble to the (n, d)-major layout.
                    scr = apool.tile([128, 768], F32, tag="scr", name="scr")
                    for h in range(HL):
                        nc.vector.transpose(
                            scr[32 * h : 32 * h + 32, :], acc[32 * h : 32 * h + 32, :]
                        )
                    rec = apool.tile([128, 24], F32, tag="rec", name="rec")
                    recx = apool.tile([128, 24 * DH], F32, tag="recx", name="recx")
                    abf = apool.tile([128, 24 * DH], BF16, tag="abf", name="abf")
                    for h in range(HL):
                        hp_ = slice(32 * h, 32 * h + 32)
                        s4 = scr[hp_, :].rearrange("p (t c j) -> p t c j", t=6, c=4)
                        nc.vector.reciprocal(
                            rec[hp_, :].rearrange("p (t c) -> p t c", t=6), s4[:, :, :, 16]
                        )
                        nc.vector.tensor_copy(
                            recx[hp_, :].rearrange("p (tc d) -> p tc d", tc=24),
                            rec[hp_, :].rearrange("p (tc o) -> p tc o", o=1).broadcast_to((32, 24, DH)),
                        )
                        # the divide itself on GpSimd (idle engine)
                        nc.gpsimd.tensor_mul(abf[hp_, :], s4[:, :, :, 0:DH], recx[hp_, :])
                        # scatter the window's (n, d)-flat piece into the
                        # channel-aligned bounce chunks (f = ((t*128+p)*16+d))
                        a4 = abf[hp_, :].rearrange("p (t c d) -> p t c d", t=6, c=4)
                        if st == 0:
                            nc.sync.dma_start(
                                bounceA[h][0:8192].rearrange("(t c a d) -> a t c d", t=4, c=4, a=32),
                                a4[:, 0:4],
                            )
                            nc.sync.dma_start(
                                bounceA[h][8192:9216].rearrange("(c a d) -> a c d", c=2, a=32),
                                a4[:, 4, 0:2],
                            )
                            nc.sync.dma_start(
                                bounceB[h][0:1024].rearrange("(c a d) -> a c d", c=2, a=32),
                                a4[:, 4, 2:4],
                            )
                            nc.sync.dma_start(
                                bounceB[h][1024:3072].rearrange("(c a d) -> a c d", c=4, a=32),
                                a4[:, 5],
                            )
                        elif st == 1:
                            nc.sync.dma_start(
                                bounceB[h][3072:9216].rearrange("(t c a d) -> a t c d", t=3, c=4, a=32),
                                a4[:, 0:3],
                            )
                            nc.sync.dma_start(
                                bounceC[h][0:6144].rearrange("(t c a d) -> a t c d", t=3, c=4, a=32),
                                a4[:, 3:6],
                            )
                        else:
                            nc.sync.dma_start(
                                bounceC[h][6144:18432].rearrange("(t c a d) -> a t c d", t=6, c=4, a=32),
                                a4[:, :],
                            )
                    gin, gout = (
                        (bounceA, gathA) if st == 0
                        else (bounceB, gathB) if st == 1
                        else (bounceC, gathC)
                    )
                    nc.gpsimd.collective_compute(
                        "AllGather",
                        mybir.AluOpType.bypass,
                        replica_groups=[[0, 1], [2, 3], [4, 5], [6, 7]],
                        ins=[gin[:].rearrange("h e -> (h e)").opt()],
                        outs=[gout[:].opt()],
                    )
                    # pull the gathered A/B channel chunks into SBUF early so
                    # only chunk C's loads sit in the tail
                    if st < 2:
                        for h in range(HL):
                            for r in range(2):
                                base = 64 * r + 16 * h + 4 * st
                                nc.sync.dma_start(
                                    g_sb[base : base + 4, :],
                                    gout[r, 9216 * h : 9216 * (h + 1)].rearrange(
                                        "(c n) -> c n", c=4
                                    ),
                                )

                # prologue: first logits slab + exps start the ACT/DVE stream
                # while the PE chews conv pass 2 and the v^T build below
                pend = (0, 0) + emit_slab(0, 0)

                # ---- conv pass 2 (init | v), overlapped with the first exps ----
                with (
                    tc.tile_pool(name="stage2", bufs=2) as stage2,
                    tc.tile_pool(name="ps_c2", bufs=2, space="PSUM") as ps_c2,
                ):
                    xp2_bf = xp_bf[:].rearrange("p (y x) -> p y x", y=50)
                    for j in range(6):
                        p = ps_c2.tile([128, 384], F32, tag="conv2", name="conv2")
                        for t in range(9):
                            dy, dx = t // 3, t % 3
                            rhs = xp2_bf[:, 8 * j + dy : 8 * j + dy + 8, dx : dx + 48]
                            nc.tensor.matmul(
                                p[:],
                                wc_bf[:, t * 256 + 128 : t * 256 + 256],
                                rhs,
                                start=(t == 0),
                                stop=(t == 8),
                            )
                        sl = slice(384 * j, 384 * (j + 1))
                        ist = stage2.tile([64, 384], F32, tag="ist", name="ist")
                        nc.vector.tensor_copy(ist[:, :], p[0:64, :])
                        nc.sync.dma_start(out_ext[0:64, sl], ist[:, :])
                        nc.vector.tensor_copy(v_sb[64:128, sl], p[64:128, :])
                    for h in range(HL):
                        nc.sync.dma_start(
                            vv_sb[32 * h : 32 * h + 16, :],
                            v_sb[64 + DH * h : 64 + DH * (h + 1), :],
                        )

                # ---- v^T via PE transpose ----
                with tc.tile_pool(name="ps_tp", bufs=2, space="PSUM") as ps_tp:
                    for h in range(HL):
                        tp = ps_tp.tile([128, NT * DH], BF16, tag="tp", name="tp", bufs=2)
                        for t in range(NT):
                            nc.tensor.matmul(
                                tp[:, DH * t : DH * (t + 1)],
                                vv_sb[32 * h : 32 * h + DH, 128 * t : 128 * (t + 1)],
                                id_bf[32 * h : 32 * h + DH, :],
                                is_transpose=True,
                                start=True,
                                stop=True,
                                tile_position=(32 * h, 0),
                            )
                        dst = vt_sb[:].rearrange("p (h t c) -> p h t c", h=HL, t=NT)
                        nc.vector.tensor_copy(
                            dst[:, h, :, 0:DH],
                            tp[:].rearrange("p (t d) -> p t d", t=NT),
                        )

                def emit_heater_burst(L, n=9):
                    # Dependency-free back-to-back zero overwrites: a >3.4us
                    # gapless PE burst flips the HAM clock gate to K=8/8
                    # (2.4 GHz). start=True never reads PSUM; the slab's real
                    # logits matmuls overwrite the region right after.
                    for k in range(n):
                        nc.tensor.matmul(
                            L[0:17, 512 * (k % 3) : 512 * (k % 3) + 512],
                            zrow[0:1, 0:17],
                            zrow[0:1, 0:512],
                            start=True,
                            stop=True,
                            skip_group_check=True,
                        )

                # ---- main software-pipelined slab loop ----
                with tc.tile_pool(name="ps_acc", bufs=1, space="PSUM") as ps_acc:
                    acc = ps_acc.tile([128, 768], F32, tag="acc", name="acc", bufs=1)
                    # init the never-matmul'd gap rows once (transposes read
                    # them; they stay zero across windows)
                    nc.vector.memset(acc[:], 0.0)
                    for sp in range(1, 3 * NT + 1):
                        if sp < 3 * NT:
                            st, mt = divmod(sp, NT)
                            # heat at the start and at window boundaries
                            heat = mt in (1, 10) or (st == 0 and mt == 2)
                            cur = (st, mt) + emit_slab(st, mt, heat=heat)
                        else:
                            cur = None
                        pst, pmt, pes, pfasts = pend
                        emit_attnv(acc, pst, pmt, pes, pfasts)
                        if pmt == NT - 1:
                            emit_finalize(acc, pst)
                        pend = cur

            # ---------------- 1x1 out conv (accumulated per gather arrival) ----------------
            with (
                tc.tile_pool(name="ps_o", bufs=6, space="PSUM") as ps_o,
                tc.tile_pool(name="ost", bufs=3) as ost,
            ):
                ocs = []
                for c in range(6):
                    oc_t = ps_o.tile([64, 384], F32, tag=f"oc{c}", name=f"oc{c}", bufs=1)
                    ocs.append(oc_t)
                for j in range(2):  # head pair (2j, 2j+1): channels 32j & 64+32j
                    for h in (2 * j, 2 * j + 1):
                        for r in range(2):
                            base = 64 * r + 16 * h
                            nc.sync.dma_start(
                                g_sb[base + 8 : base + 16, :],
                                gathC[r, 18432 * h : 18432 * (h + 1)].rearrange("(c n) -> c n", c=8),
                            )
                    for (bi, base) in enumerate((32 * j, 64 + 32 * j)):
                        for c in range(6):
                            nc.tensor.matmul(
                                ocs[c][:],
                                wo_bf[base : base + 32, :],
                                g_sb[base : base + 32, 384 * c : 384 * (c + 1)],
                                start=(j == 0 and bi == 0),
                                stop=(j == 1 and bi == 1),
                                tile_position=(base, 0),
                                skip_group_check=True,
                            )
                            if j == 1 and bi == 1:
                                ot = ost.tile([64, 384], F32, tag="ot")
                                nc.vector.tensor_copy(ot[:], ocs[c][:])
                                nc.sync.dma_start(out_ext[64:128, 384 * c : 384 * (c + 1)], ot[:])

    nc.compile()
    return nc


_NC_CACHE = None


def _get_nc(DEBUG=False):
    global _NC_CACHE
    if _NC_CACHE is None:
        _NC_CACHE = build(DEBUG)
    return _NC_CACHE


def _shard_inputs(x, w_init, w_qkv, w_out):
    in_maps = []
    taps = lambda w: w.reshape(w.shape[0], 128, 9)  # (O, I, 3, 3) -> (O, I, 9)
    wi, wq, wo = taps(w_init), taps(w_qkv), w_out[:, :, 0, 0]
    for c in range(8):
        b, g = c // 2, c % 2
        s = slice(64 * g, 64 * (g + 1))
        # mt0 cols: [k_h(16) | q_h(16)] x 4 heads; mt1 cols: [init 64 | v 64];
        # delivered as (128, 9*256) = [ci, (tap, col)] for one contiguous DMA
        wc = np.empty((9, 128, 256), np.float32)
        for h in range(4):
            wc[:, :, 32 * h : 32 * h + 16] = wq[128 + 64 * g + 16 * h : 128 + 64 * g + 16 * (h + 1)].transpose(2, 1, 0)
            wc[:, :, 32 * h + 16 : 32 * h + 32] = wq[64 * g + 16 * h : 64 * g + 16 * (h + 1)].transpose(2, 1, 0)
        wc[:, :, 128:192] = wi[s].transpose(2, 1, 0)
        wc[:, :, 192:256] = wq[256 + 64 * g : 256 + 64 * (g + 1)].transpose(2, 1, 0)
        wc = np.ascontiguousarray(wc.transpose(1, 0, 2).reshape(128, 9 * 256))
        in_maps.append(
            {
                "x": np.ascontiguousarray(x[b], np.float32),
                "wc": np.ascontiguousarray(wc),
                "wo": np.ascontiguousarray(wo[s].T, dtype=np.float32),
            }
        )
    return in_maps


def kernel(x, w_init, w_qkv, w_out, _trace=False, _debug=False):
    nc = _get_nc(_debug)
    in_maps = _shard_inputs(
        np.asarray(x, np.float32),
        np.asarray(w_init, np.float32),
        np.asarray(w_qkv, np.float32),
        np.asarray(w_out, np.float32),
    )
    res = run_bass_kernel_spmd(nc, in_maps, core_ids=list(range(8)), trace=_trace)
    full = np.empty((4, 256, 48, 48), np.float32)
    for c in range(8):
        b, g = c // 2, c % 2
        o = res.results[c]["out"].reshape(128, 48, 48)
        full[b, 64 * g : 64 * (g + 1)] = o[0:64]
        full[b, 128 + 64 * g : 128 + 64 * (g + 1)] = o[64:128]
    if _trace:
        return full, res
    return full


# revision 25
# speedup vs baseline: 1.1043x; 1.1043x over previous
"""Trainium2 Bass kernel for nn_AConvCircular2D (4x128x48x48, 8 heads, dk=dv=128).

Sharding: core c handles batch b = c//2 and head-group g = c%2 (heads 4g..4g+3).
Per core:
  - 3x3 circular convs (init 64ch + q 64ch + k 64ch + v 64ch) as 9-tap matmul
    accumulation over a circularly padded x in SBUF (all bf16, fp32 accum).
  - attention, pair-interleaved, n-window (768) outer / m-tile inner:
      slab (pi, mt) = logitsT for heads {2pi, 2pi+1} on PE row-bands 32h;
      exp on ACT (pair 0) or DVE int16-bf16-bits fast-exp (pair 1), running
      concurrently; attn^T[d, n-window] += vt[mt]^T @ E with the small v^T
      chunk [128, 17] stationary (ones column -> softmax sums row).
      attnv is software-pipelined one mt behind logits so the PE never
      blocks on the exp engines. Conv pass 2 (init|v) and the v^T build are
      emitted AFTER the first logits slab so the exp stream starts early.
  - per-window finalize: DVE 32x32 block transposes PSUM->SBUF (scrambled
    layout), softmax divide (DVE recip + GpSimd mul), DMAs into
    channel-aligned DRAM chunks; one AllGather per chunk {w0: ch 0:4,
    w1: ch 4:8, w2: ch 8:16} across the core pair -> only the last is
    tail-exposed.
  - 1x1 out-conv (64 out channels per core).
Output per core: (128, 2304) f32 = [64 init-conv channels | 64 attn-conv channels].
"""

import sys

sys.path.insert(0, "/opt/trn_rl_repo")

import numpy as np

import concourse.bass as bass  # noqa: F401
import concourse.mybir as mybir
from concourse import bacc, tile
from concourse.bass_utils import run_bass_kernel_spmd

F32 = mybir.dt.float32
BF16 = mybir.dt.bfloat16
AF = mybir.ActivationFunctionType

H = 48
W = 48
N = H * W            # 2304
NT = N // 128        # 18 n-tiles / m-tiles of 128
DH = 16              # per-head dim
HL = 4               # heads per core
SCALE = DH ** -0.5   # 0.25
I16 = mybir.dt.int16

# DVE fast-exp: bf16 bit pattern of 2^(SCALE*log2e*x) built as int16
# bits = trunc(x * EXP_A + EXP_B); softmax cancels the constant-factor part
# of the approximation error, leaving only mantissa-curvature (~1.4% on a
# fully-converted softmax row; diluted by the 50/50 ACT/DVE pair split).
EXP_A = float(SCALE * 1.4426950408889634 * 128.0)
EXP_B = 16250.5


def build(DEBUG=False):
    nc = bacc.Bacc("TRN2", target_bir_lowering=False, debug=False)

    x_ext = nc.declare_dram_parameter("x", [128, H, W], F32, isOutput=False)
    wc_ext = nc.declare_dram_parameter("wc", [128, 9 * 256], F32, isOutput=False)
    wo_ext = nc.declare_dram_parameter("wo", [128, 64], F32, isOutput=False)
    out_ext = nc.declare_dram_parameter("out", [128, N], F32, isOutput=True)

    with tile.TileContext(nc) as tc:
        with (
            tc.tile_pool(name="const", bufs=1) as cp,
            tc.tile_pool(name="dram", bufs=1, space="DRAM") as dram,
        ):
            xp_bf = cp.tile([128, 50 * 50], BF16)    # circular-padded x
            wc_bf = cp.tile([128, 9 * 256], BF16)    # conv weights, tap-major
            wo_bf = cp.tile([128, 64], BF16)         # out-conv weights (in,out)
            qm_sb = cp.tile([128, N], BF16)          # [k_h | q_h] per 32-block
            kz_sb = cp.tile([128, N], BF16)          # [0 | k_h] per 32-block
            v_sb = cp.tile([128, N], BF16)           # rows 64..127 hold v (conv order)
            vv_sb = cp.tile([128, N], BF16)          # v_h at base 32h
            vt_sb = cp.tile([128, HL * NT * 17], BF16)  # v^T + ones col per m-tile
            g_sb = cp.tile([128, N], BF16)           # gathered attn channels
            id_bf = cp.tile([128, 16], BF16)  # identity replicated per 16-row block

            bounceA = dram.tile([HL, 9216], BF16)    # channels 0:4 per head (after w0)
            bounceB = dram.tile([HL, 9216], BF16)    # channels 4:8 (after w1)
            bounceC = dram.tile([HL, 18432], BF16)   # channels 8:16 (after w2)
            gathA = dram.tile([2, HL * 9216], BF16)
            gathB = dram.tile([2, HL * 9216], BF16)
            gathC = dram.tile([2, HL * 18432], BF16)

            # ---------------- load + prep + conv pass 1 (q|k) ----------------
            with (
                tc.tile_pool(name="stage", bufs=2) as stage,
                tc.tile_pool(name="ps_a", bufs=3, space="PSUM") as ps_a,
            ):
                # PE warm-up: dummy matmuls during input DMA to lift the HAM
                # clock gate to 2.4 GHz before the conv streams
                warm = stage.tile([128, 512], BF16, tag="warm")
                nc.vector.memset(warm[:], 0.0)
                wps = ps_a.tile([128, 512], F32, tag="wps", bufs=1)
                for _ in range(16):
                    nc.tensor.matmul(wps[:], warm[:, 0:128], warm[:], start=True, stop=True)
                # preload the exp ACT table set off the critical path
                wexp = stage.tile([128, 16], BF16, tag="wexp")
                nc.scalar.activation(wexp[:], warm[:, 0:16], AF.Exp)

                x_f32 = stage.tile([128, 2304], F32, tag="xstage")
                nc.sync.dma_start(x_f32[:], x_ext[:].rearrange("p y x -> p (y x)"))
                x2 = x_f32[:].rearrange("p (y x) -> p y x", y=48)
                xp2b = xp_bf[:].rearrange("p (y x) -> p y x", y=50)
                # halo build on DVE with fused f32->bf16 cast
                nc.vector.tensor_copy(xp2b[:, 1:49, 1:49], x2[:, :, :])
                nc.vector.tensor_copy(xp2b[:, 1:49, 0:1], x2[:, :, 47:48])
                nc.vector.tensor_copy(xp2b[:, 1:49, 49:50], x2[:, :, 0:1])
                nc.vector.tensor_copy(xp2b[:, 0:1, 1:49], x2[:, 47:48, :])
                nc.vector.tensor_copy(xp2b[:, 49:50, 1:49], x2[:, 0:1, :])
                nc.vector.tensor_copy(xp2b[:, 0:1, 0:1], x2[:, 47:48, 47:48])
                nc.vector.tensor_copy(xp2b[:, 0:1, 49:50], x2[:, 47:48, 0:1])
                nc.vector.tensor_copy(xp2b[:, 49:50, 0:1], x2[:, 0:1, 47:48])
                nc.vector.tensor_copy(xp2b[:, 49:50, 49:50], x2[:, 0:1, 0:1])

                wc_f32 = stage.tile([128, 9 * 256], F32, tag="wcstage")
                nc.sync.dma_start(wc_f32[:], wc_ext[:])
                nc.vector.tensor_copy(wc_bf[:], wc_f32[:])

                wo_f32 = stage.tile([128, 64], F32, tag="wo")
                nc.sync.dma_start(wo_f32[:], wo_ext[:])
                nc.vector.tensor_copy(wo_bf[:], wo_f32[:])

                nc.gpsimd.memset(id_bf[:], 0.0)
                nc.gpsimd.affine_select(
                    out=id_bf[0:16, :],
                    in_=id_bf[0:16, :],
                    compare_op=mybir.AluOpType.not_equal,
                    fill=1.0,
                    base=0,
                    pattern=[[-1, 16]],
                    channel_multiplier=1,
                )
                for blk in (2, 4, 6):
                    nc.sync.dma_start(id_bf[16 * blk : 16 * blk + 16, :], id_bf[0:16, :])
                nc.gpsimd.memset(vt_sb[:], 1.0)
                nc.gpsimd.memset(kz_sb[:], 0.0)

                # conv pass 1: [k|q] channels
                xp2_bf = xp_bf[:].rearrange("p (y x) -> p y x", y=50)
                for j in range(6):  # n-chunks of 384 (8 rows)
                    p = ps_a.tile([128, 384], F32, tag="conv")
                    for t in range(9):
                        dy, dx = t // 3, t % 3
                        rhs = xp2_bf[:, 8 * j + dy : 8 * j + dy + 8, dx : dx + 48]
                        nc.tensor.matmul(
                            p[:],
                            wc_bf[:, t * 256 : t * 256 + 128],
                            rhs,
                            start=(t == 0),
                            stop=(t == 8),
                        )
                    nc.vector.tensor_copy(qm_sb[:, 384 * j : 384 * (j + 1)], p[:, :])

                # shift k into the [0|k] stationary layout
                for h in range(HL):
                    nc.sync.dma_start(
                        kz_sb[32 * h + 16 : 32 * h + 32, :], qm_sb[32 * h : 32 * h + 16, :]
                    )

            # ---------------- attention + conv pass 2, pipelined ----------------
            with (
                tc.tile_pool(name="epool", bufs=3) as epool,
                tc.tile_pool(name="apool", bufs=2) as apool,
                tc.tile_pool(name="ps_log", bufs=2, space="PSUM") as ps_log,
            ):
                def emit_slab(st, mt):
                    """Emit both pairs' logits + exps for (st, mt); return E info."""
                    nb = 768 * st
                    es, fasts = [], []
                    for pi in range(2):
                        h0, h1 = 2 * pi, 2 * pi + 1
                        L = ps_log.tile([128, 1536], F32, tag="L", name="L")
                        kz0 = kz_sb[32 * h0 : 32 * h0 + 32, 128 * mt : 128 * (mt + 1)]
                        kz1 = kz_sb[32 * h1 : 32 * h1 + 32, 128 * mt : 128 * (mt + 1)]
                        # issue order keeps the two bank-1 chunks (cols 512:768
                        # and 768:1024) temporally disjoint under strip concurrency
                        for (i, kzh, cols, ns, w_) in (
                            (0, kz0, 512, 512, 256),
                            (1, kz1, 1024, 256, 512),
                            (0, kz0, 0, 0, 512),
                            (1, kz1, 768, 0, 256),
                        ):
                            h = 2 * pi + i
                            nc.tensor.matmul(
                                L[:, cols : cols + w_],
                                kzh,
                                qm_sb[32 * h : 32 * h + 32, nb + ns : nb + ns + w_],
                                start=True,
                                stop=True,
                                tile_position=(32 * h, 0),
                            )
                        # pair 0's exp on ACT; pair 1's on DVE via the int16
                        # bf16-bits fast-exp (concurrent engines)
                        e = epool.tile([128, 1536], BF16, tag="E", name="E")
                        if pi == 0:
                            nc.scalar.activation(e[:], L[:], AF.Exp, scale=SCALE)
                        else:
                            nc.vector.tensor_scalar(
                                e[:].bitcast(I16), L[:], EXP_A, EXP_B,
                                op0=mybir.AluOpType.mult,
                                op1=mybir.AluOpType.add,
                            )
                        es.append(e)
                    return es, fasts

                def emit_attnv(acc, st, mt, es, fasts):
                    first, last = (mt == 0), (mt == NT - 1)
                    for pi in range(2):
                        e = es[pi]
                        for i in range(2):
                            h = 2 * pi + i
                            vt = vt_sb[:, (h * NT + mt) * 17 : (h * NT + mt) * 17 + 17]
                            for (c0, w_) in ((0, 512), (512, 256)):
                                rhs = e[:, 768 * i + c0 : 768 * i + c0 + w_]
                                nc.tensor.matmul(
                                    acc[32 * h : 32 * h + 17, c0 : c0 + w_],
                                    vt,
                                    rhs,
                                    start=first,
                                    stop=last,
                                    tile_position=(0, 32 * h),
                                    skip_group_check=True,
                                )

                def emit_finalize(acc, st):
                    # One 32x32-block DVE transpose per head band, PSUM->SBUF:
                    # scr[32h+a, 128t+32c+b] = acc[32h+b, 128t+32c+a], i.e.
                    # partition a = n%32, col b = d-index (0:16 data, 16 sums).
                    # The bounce DMAs' APs unscramble to the (n, d)-major layout.
                    scr = apool.tile([128, 768], F32, tag="scr", name="scr")
                    for h in range(HL):
                        nc.vector.transpose(
                            scr[32 * h : 32 * h + 32, :], acc[32 * h : 32 * h + 32, :]
                        )
                    rec = apool.tile([128, 24], F32, tag="rec", name="rec")
                    recx = apool.tile([128, 24 * DH], F32, tag="recx", name="recx")
                    abf = apool.tile([128, 24 * DH], BF16, tag="abf", name="abf")
                    for h in range(HL):
                        hp_ = slice(32 * h, 32 * h + 32)
                        s4 = scr[hp_, :].rearrange("p (t c j) -> p t c j", t=6, c=4)
                        nc.vector.reciprocal(
                            rec[hp_, :].rearrange("p (t c) -> p t c", t=6), s4[:, :, :, 16]
                        )
                        nc.vector.tensor_copy(
                            recx[hp_, :].rearrange("p (tc d) -> p tc d", tc=24),
                            rec[hp_, :].rearrange("p (tc o) -> p tc o", o=1).broadcast_to((32, 24, DH)),
                        )
                        # the divide itself on GpSimd (idle engine)
                        nc.gpsimd.tensor_mul(abf[hp_, :], s4[:, :, :, 0:DH], recx[hp_, :])
                        # scatter the window's (n, d)-flat piece into the
                        # channel-aligned bounce chunks (f = ((t*128+p)*16+d))
                        a4 = abf[hp_, :].rearrange("p (t c d) -> p t c d", t=6, c=4)
                        if st == 0:
                            nc.sync.dma_start(
                                bounceA[h][0:8192].rearrange("(t c a d) -> a t c d", t=4, c=4, a=32),
                                a4[:, 0:4],
                            )
                            nc.sync.dma_start(
                                bounceA[h][8192:9216].rearrange("(c a d) -> a c d", c=2, a=32),
                                a4[:, 4, 0:2],
                            )
                            nc.sync.dma_start(
                                bounceB[h][0:1024].rearrange("(c a d) -> a c d", c=2, a=32),
                                a4[:, 4, 2:4],
                            )
                            nc.sync.dma_start(
                                bounceB[h][1024:3072].rearrange("(c a d) -> a c d", c=4, a=32),
                                a4[:, 5],
                            )
                        elif st == 1:
                            nc.sync.dma_start(
                                bounceB[h][3072:9216].rearrange("(t c a d) -> a t c d", t=3, c=4, a=32),
                                a4[:, 0:3],
                            )
                            nc.sync.dma_start(
                                bounceC[h][0:6144].rearrange("(t c a d) -> a t c d", t=3, c=4, a=32),
                                a4[:, 3:6],
                            )
                        else:
                            nc.sync.dma_start(
                                bounceC[h][6144:18432].rearrange("(t c a d) -> a t c d", t=6, c=4, a=32),
                                a4[:, :],
                            )
                    gin, gout = (
                        (bounceA, gathA) if st == 0
                        else (bounceB, gathB) if st == 1
                        else (bounceC, gathC)
                    )
                    nc.gpsimd.collective_compute(
                        "AllGather",
                        mybir.AluOpType.bypass,
                        replica_groups=[[0, 1], [2, 3], [4, 5], [6, 7]],
                        ins=[gin[:].rearrange("h e -> (h e)").opt()],
                        outs=[gout[:].opt()],
                    )
                    # pull the gathered A/B channel chunks into SBUF early so
                    # only chunk C's loads sit in the tail
                    if st < 2:
                        for h in range(HL):
                            for r in range(2):
                                base = 64 * r + 16 * h + 4 * st
                                nc.sync.dma_start(
                                    g_sb[base : base + 4, :],
                                    gout[r, 9216 * h : 9216 * (h + 1)].rearrange(
                                        "(c n) -> c n", c=4
                                    ),
                                )

                # prologue: first logits slab + exps start the ACT/DVE stream
                # while the PE chews conv pass 2 and the v^T build below
                pend = (0, 0) + emit_slab(0, 0)

                # ---- conv pass 2 (init | v), overlapped with the first exps ----
                with (
                    tc.tile_pool(name="stage2", bufs=2) as stage2,
                    tc.tile_pool(name="ps_c2", bufs=2, space="PSUM") as ps_c2,
                ):
                    xp2_bf = xp_bf[:].rearrange("p (y x) -> p y x", y=50)
                    for j in range(6):
                        p = ps_c2.tile([128, 384], F32, tag="conv2", name="conv2")
                        for t in range(9):
                            dy, dx = t // 3, t % 3
                            rhs = xp2_bf[:, 8 * j + dy : 8 * j + dy + 8, dx : dx + 48]
                            nc.tensor.matmul(
                                p[:],
                                wc_bf[:, t * 256 + 128 : t * 256 + 256],
                                rhs,
                                start=(t == 0),
                                stop=(t == 8),
                            )
                        sl = slice(384 * j, 384 * (j + 1))
                        ist = stage2.tile([64, 384], F32, tag="ist", name="ist")
                        nc.vector.tensor_copy(ist[:, :], p[0:64, :])
                        nc.sync.dma_start(out_ext[0:64, sl], ist[:, :])
                        nc.vector.tensor_copy(v_sb[64:128, sl], p[64:128, :])
                    for h in range(HL):
                        nc.sync.dma_start(
                            vv_sb[32 * h : 32 * h + 16, :],
                            v_sb[64 + DH * h : 64 + DH * (h + 1), :],
                        )

                # ---- v^T via PE transpose ----
                with tc.tile_pool(name="ps_tp", bufs=2, space="PSUM") as ps_tp:
                    for h in range(HL):
                        tp = ps_tp.tile([128, NT * DH], BF16, tag="tp", name="tp", bufs=2)
                        for t in range(NT):
                            nc.tensor.matmul(
                                tp[:, DH * t : DH * (t + 1)],
                                vv_sb[32 * h : 32 * h + DH, 128 * t : 128 * (t + 1)],
                                id_bf[32 * h : 32 * h + DH, :],
                                is_transpose=True,
                                start=True,
                                stop=True,
                                tile_position=(32 * h, 0),
                            )
                        dst = vt_sb[:].rearrange("p (h t c) -> p h t c", h=HL, t=NT)
                        nc.vector.tensor_copy(
                            dst[:, h, :, 0:DH],
                            tp[:].rearrange("p (t d) -> p t d", t=NT),
                        )


                # ---- main software-pipelined slab loop ----
                with tc.tile_pool(name="ps_acc", bufs=1, space="PSUM") as ps_acc:
                    acc = ps_acc.tile([128, 768], F32, tag="acc", name="acc", bufs=1)
                    # init the never-matmul'd gap rows once (transposes read
                    # them; they stay zero across windows)
                    nc.vector.memset(acc[:], 0.0)
                    for sp in range(1, 3 * NT + 1):
                        if sp < 3 * NT:
                            st, mt = divmod(sp, NT)
                            cur = (st, mt) + emit_slab(st, mt)
                        else:
                            cur = None
                        pst, pmt, pes, pfasts = pend
                        emit_attnv(acc, pst, pmt, pes, pfasts)
                        if pmt == NT - 1:
                            emit_finalize(acc, pst)
                        pend = cur

            # ---------------- 1x1 out conv (accumulated per gather arrival) ----------------
            with (
                tc.tile_pool(name="ps_o", bufs=6, space="PSUM") as ps_o,
                tc.tile_pool(name="ost", bufs=3) as ost,
            ):
                ocs = []
                for c in range(6):
                    oc_t = ps_o.tile([64, 384], F32, tag=f"oc{c}", name=f"oc{c}", bufs=1)
                    ocs.append(oc_t)
                for j in range(2):  # head pair (2j, 2j+1): channels 32j & 64+32j
                    for h in (2 * j, 2 * j + 1):
                        for r in range(2):
                            base = 64 * r + 16 * h
                            nc.sync.dma_start(
                                g_sb[base + 8 : base + 16, :],
                                gathC[r, 18432 * h : 18432 * (h + 1)].rearrange("(c n) -> c n", c=8),
                            )
                    for (bi, base) in enumerate((32 * j, 64 + 32 * j)):
                        for c in range(6):
                            nc.tensor.matmul(
                                ocs[c][:],
                                wo_bf[base : base + 32, :],
                                g_sb[base : base + 32, 384 * c : 384 * (c + 1)],
                                start=(j == 0 and bi == 0),
                                stop=(j == 1 and bi == 1),
                                tile_position=(base, 0),
                                skip_group_check=True,
                            )
                            if j == 1 and bi == 1:
                                ot = ost.tile([64, 384], F32, tag="ot")
                                nc.vector.tensor_copy(ot[:], ocs[c][:])
                                nc.sync.dma_start(out_ext[64:128, 384 * c : 384 * (c + 1)], ot[:])

    nc.compile()
    return nc


_NC_CACHE = None


def _get_nc(DEBUG=False):
    global _NC_CACHE
    if _NC_CACHE is None:
        _NC_CACHE = build(DEBUG)
    return _NC_CACHE


def _shard_inputs(x, w_init, w_qkv, w_out):
    in_maps = []
    taps = lambda w: w.reshape(w.shape[0], 128, 9)  # (O, I, 3, 3) -> (O, I, 9)
    wi, wq, wo = taps(w_init), taps(w_qkv), w_out[:, :, 0, 0]
    for c in range(8):
        b, g = c // 2, c % 2
        s = slice(64 * g, 64 * (g + 1))
        # mt0 cols: [k_h(16) | q_h(16)] x 4 heads; mt1 cols: [init 64 | v 64];
        # delivered as (128, 9*256) = [ci, (tap, col)] for one contiguous DMA
        wc = np.empty((9, 128, 256), np.float32)
        for h in range(4):
            wc[:, :, 32 * h : 32 * h + 16] = wq[128 + 64 * g + 16 * h : 128 + 64 * g + 16 * (h + 1)].transpose(2, 1, 0)
            wc[:, :, 32 * h + 16 : 32 * h + 32] = wq[64 * g + 16 * h : 64 * g + 16 * (h + 1)].transpose(2, 1, 0)
        wc[:, :, 128:192] = wi[s].transpose(2, 1, 0)
        wc[:, :, 192:256] = wq[256 + 64 * g : 256 + 64 * (g + 1)].transpose(2, 1, 0)
        wc = np.ascontiguousarray(wc.transpose(1, 0, 2).reshape(128, 9 * 256))
        in_maps.append(
            {
                "x": np.ascontiguousarray(x[b], np.float32),
                "wc": np.ascontiguousarray(wc),
                "wo": np.ascontiguousarray(wo[s].T, dtype=np.float32),
            }
        )
    return in_maps


def kernel(x, w_init, w_qkv, w_out, _trace=False, _debug=False):
    nc = _get_nc(_debug)
    in_maps = _shard_inputs(
        np.asarray(x, np.float32),
        np.asarray(w_init, np.float32),
        np.asarray(w_qkv, np.float32),
        np.asarray(w_out, np.float32),
    )
    res = run_bass_kernel_spmd(nc, in_maps, core_ids=list(range(8)), trace=_trace)
    full = np.empty((4, 256, 48, 48), np.float32)
    for c in range(8):
        b, g = c // 2, c % 2
        o = res.results[c]["out"].reshape(128, 48, 48)
        full[b, 64 * g : 64 * (g + 1)] = o[0:64]
        full[b, 128 + 64 * g : 128 + 64 * (g + 1)] = o[64:128]
    if _trace:
        return full, res
    return full
